# revision 1
# baseline (speedup 1.0000x reference)
"""GCNConv (N=100000, E=1600000, C=128) on 8 trn2 NeuronCores.

Sharding strategy (node-parallel, per the hint): destination nodes are
partitioned across the 8 cores, load-balanced (LPT bin-packing) into
128-row dest tiles. Edge routing is done on host as part of sharding:
edges are bucketed by destination tile and the per-edge source feature
rows (the "gathered source features" of the hint's all-to-all) are
materialized as a dest-sorted bf16 stream per core, from the
dis[col]-prescaled table x'' = diag(1/sqrt(deg)) @ x. The device then
does all the math: the segment_sum over each destination's messages
(PE selection-matmuls accumulating in PSUM), the W transform, and the
dis[row] output scaling.

Why no device-side per-edge gather: every dynamic-indexing mechanism on
trn2 (SWDGE indirect DMA, InstDMAGatherAnt, InstAPGather) was measured
at ~50 ns per row/descriptor per core (Q7 ucode rate), i.e. >10 ms for
1.7M edges -- 40x slower than streaming the routed messages at HBM rate.

Device pipeline per dest tile t (128 dests, K=17 chunks of 128 messages):
  msgs tile [128 msg, K*128 feat] <- one contiguous 557KB DMA (HWDGE)
  SelT[m, k*128+d] = (dlocal[m,k] == d)     one whole-tile DVE is_equal
  for chunk c: psum_sT[feat, dest] += msgs_c.T @ SelT_c     # PE, fp32
  sT -> SBUF bf16 (ACT copy)
  psum_out[dest, feat_out] = sT.T @ W                        # PE
  out_t = psum_out * disout   (ACT) -> DMA to HBM

Measured: ~250 us per pass on 8 cores (message stream 437MB bf16 at
~2.9TB/s aggregate + DVE sel builds, fully overlapped); output rel err
vs fp32 reference ~2.9e-3 (bf16 messages/weights, fp32 accumulation).
"""
import math

import numpy as np
import ml_dtypes

import concourse.bacc as bacc
import concourse.tile as tile
from concourse import mybir
from concourse.bass import AP
from concourse.bass_utils import run_bass_kernel_spmd

N_CORES = 8
P = 128

BF16 = ml_dtypes.bfloat16


def build_nc(n_tiles: int, K: int, repeat: int = 1, msgs_tiles=None,
             sel_mode='multi', copy_eng='scalar', disout_eng='scalar',
             gp_frac=0, bufs=None):
    """Build the SPMD Bass kernel: n_tiles dest tiles per core, K chunks of
    128 messages per tile.

    repeat>1 wraps the tile loop in a hardware For_i (idempotent re-run;
    timing only). msgs_tiles (timing only) shrinks the msgs input to that
    many tiles, read as msgs[t % msgs_tiles] -- same device work, tiny
    host->device transfer."""
    nc = bacc.Bacc("TRN2", target_bir_lowering=False, debug=False)
    T = n_tiles
    f32 = mybir.dt.float32
    bf16 = mybir.dt.bfloat16

    MT = msgs_tiles if msgs_tiles is not None else T
    b = {"msgp": 4, "selp": 8, "sTp": 3, "outp": 3, "psA": 4, "psB": 2}
    if bufs:
        b.update(bufs)
    msgs = nc.dram_tensor("msgs", [MT, P, K * P], bf16, kind="ExternalInput")
    dlocal = nc.dram_tensor("dlocal", [P, T * K], bf16, kind="ExternalInput")
    disout = nc.dram_tensor("disout", [P, T], f32, kind="ExternalInput")
    w16 = nc.dram_tensor("w16", [P, P], bf16, kind="ExternalInput")
    iota = nc.dram_tensor("iota", [P, P], bf16, kind="ExternalInput")
    if sel_mode == "ts":
        dlocal32 = nc.dram_tensor("dlocal32", [P, T * K], f32, kind="ExternalInput")
    out = nc.dram_tensor("out", [T * P, P], f32, kind="ExternalOutput")

    with tile.TileContext(nc) as tc:
        with tc.tile_pool(name="const", bufs=1) as constp, \
             tc.tile_pool(name="msgp", bufs=b["msgp"]) as msgp, \
             tc.tile_pool(name="selp", bufs=b["selp"]) as selp, \
             tc.tile_pool(name="sTp", bufs=b["sTp"]) as sTp, \
             tc.tile_pool(name="outp", bufs=b["outp"]) as outp, \
             tc.tile_pool(name="psA", bufs=b["psA"], space="PSUM") as psA, \
             tc.tile_pool(name="psB", bufs=b["psB"], space="PSUM") as psB:
            w_t = constp.tile([P, P], bf16)
            nc.sync.dma_start(w_t[:], w16[:])
            iota_t = constp.tile([P, P], bf16)
            nc.sync.dma_start(iota_t[:], iota[:])
            dlocal_t = constp.tile([P, T * K], bf16)
            nc.sync.dma_start(dlocal_t[:], dlocal[:])
            disout_t = constp.tile([P, T], f32)
            nc.sync.dma_start(disout_t[:], disout[:])
            if sel_mode == "ts":
                dlocal32_t = constp.tile([P, T * K], f32)
                nc.sync.dma_start(dlocal32_t[:], dlocal32[:])

            def body():
              for t in range(T):
                m_t = msgp.tile([P, K * P], bf16, tag="m")
                nc.sync.dma_start(m_t[:], msgs[t % MT])
                ps = psA.tile([P, P], f32, tag="psA")
                if sel_mode == "multi":
                    sel_m = selp.tile([P, K * P], bf16, tag="sel")
                    sel3 = sel_m[:].rearrange("p (k f) -> p k f", k=K)
                    dl_b = dlocal_t[:, t * K:(t + 1) * K].broadcast_to([P, K, P])
                    io = iota_t[:]
                    io3 = AP(io.tensor, io.offset, [[io.ap[0][0], P], [0, K], [1, P]])
                    eng = nc.gpsimd if (gp_frac and t % gp_frac == gp_frac - 1) \
                        else nc.vector
                    eng.tensor_tensor(
                        out=sel3, in0=dl_b, in1=io3, op=mybir.AluOpType.is_equal)
                    for c in range(K):
                        nc.tensor.matmul(
                            out=ps[:],
                            lhsT=m_t[:, c * P:(c + 1) * P],
                            rhs=sel_m[:, c * P:(c + 1) * P],
                            start=(c == 0),
                            stop=(c == K - 1),
                        )
                elif sel_mode == "ts":
                    for c in range(K):
                        col = t * K + c
                        sel = selp.tile([P, P], bf16, tag="sel")
                        nc.vector.tensor_scalar(
                            out=sel[:],
                            in0=iota_t[:],
                            scalar1=dlocal32_t[:, col:col + 1],
                            scalar2=None,
                            op0=mybir.AluOpType.is_equal,
                        )
                        nc.tensor.matmul(
                            out=ps[:],
                            lhsT=m_t[:, c * P:(c + 1) * P],
                            rhs=sel[:],
                            start=(c == 0),
                            stop=(c == K - 1),
                        )
                elif sel_mode == "none":
                    for c in range(K):
                        nc.tensor.matmul(
                            out=ps[:], lhsT=m_t[:, c * P:(c + 1) * P],
                            rhs=w_t[:], start=(c == 0), stop=(c == K - 1))
                else:
                    for c in range(K):
                        col = t * K + c
                        sel = selp.tile([P, P], bf16, tag="sel")
                        nc.vector.tensor_tensor(
                            out=sel[:],
                            in0=dlocal_t[:, col:col + 1].to_broadcast([P, P]),
                            in1=iota_t[:],
                            op=mybir.AluOpType.is_equal,
                        )
                        nc.tensor.matmul(
                            out=ps[:],
                            lhsT=m_t[:, c * P:(c + 1) * P],
                            rhs=sel[:],
                            start=(c == 0),
                            stop=(c == K - 1),
                        )
                sT = sTp.tile([P, P], bf16, tag="sT")
                if copy_eng == "scalar":
                    nc.scalar.copy(out=sT[:], in_=ps[:])
                else:
                    nc.vector.tensor_copy(out=sT[:], in_=ps[:])
                ps2 = psB.tile([P, P], f32, tag="psB")
                nc.tensor.matmul(out=ps2[:], lhsT=sT[:], rhs=w_t[:],
                                 start=True, stop=True)
                o_t = outp.tile([P, P], f32, tag="o")
                if disout_eng == "scalar":
                    nc.scalar.mul(o_t[:], ps2[:], disout_t[:, t:t + 1])
                else:
                    nc.vector.tensor_scalar_mul(o_t[:], ps2[:], disout_t[:, t:t + 1])
                nc.sync.dma_start(out[t * P:(t + 1) * P, :], o_t[:])
            if repeat == 1:
                body()
            else:
                with tc.For_i(0, repeat, 1):
                    body()
    nc.compile()
    return nc


def _route(x, W, edge_index, num_nodes, n_cores=N_CORES):
    """Host-side sharding/routing. Returns (in_maps, node_of, n_tiles, K)."""
    N = int(num_nodes)
    row = np.asarray(edge_index[0], dtype=np.int64)
    col = np.asarray(edge_index[1], dtype=np.int64)
    loops = np.arange(N, dtype=np.int64)
    row = np.concatenate([row, loops])
    col = np.concatenate([col, loops])
    E = row.shape[0]

    # symmetric degree normalization (degree counted on col, as reference)
    deg = np.bincount(col, minlength=N)
    dis = np.zeros(N, dtype=np.float32)
    nz = deg > 0
    dis[nz] = 1.0 / np.sqrt(deg[nz].astype(np.float64)).astype(np.float32)

    # --- load-balanced assignment of dest nodes to (core, tile, slot) ---
    deg_in = np.bincount(row, minlength=N)  # messages per dest
    n_tiles = math.ceil(N / (n_cores * P) / 1.0)
    n_tiles = math.ceil(N / n_cores / P)          # tiles per core
    TT = n_cores * n_tiles                        # total tiles
    # LPT: biggest dests first, into least-loaded tile with free slots
    import heapq
    order = np.argsort(-deg_in, kind="stable")
    heap = [(0, tt) for tt in range(TT)]
    heapq.heapify(heap)
    slots_used = np.zeros(TT, dtype=np.int64)
    tile_of = np.empty(N, dtype=np.int64)
    slot_of = np.empty(N, dtype=np.int64)
    spill = []
    for d in order:
        while True:
            load, tt = heapq.heappop(heap)
            if slots_used[tt] < P:
                break
            spill.append((load, tt))  # full tile: drop permanently
        tile_of[d] = tt
        slot_of[d] = slots_used[tt]
        slots_used[tt] += 1
        heapq.heappush(heap, (load + int(deg_in[d]), tt))

    # edges -> tiles, then slots within tile
    gt = tile_of[row]                             # tile of each edge
    e_order = np.argsort(gt, kind="stable")
    gt_s = gt[e_order]
    counts = np.bincount(gt_s, minlength=TT)
    K = int(math.ceil(counts.max() / P))
    starts = np.zeros(TT + 1, dtype=np.int64)
    np.cumsum(counts, out=starts[1:])
    pos = np.arange(E, dtype=np.int64) - starts[gt_s]
    c_e = pos // P
    m_e = pos % P

    x16 = (np.asarray(x, dtype=np.float32) * dis[:, None]).astype(BF16)

    msgs = np.zeros((TT, P, K, P), dtype=BF16)
    msgs[gt_s, m_e, c_e, :] = x16[col[e_order]]

    dlocal = np.full((TT, K, P), 255.0, dtype=BF16)
    dlocal[gt_s, c_e, m_e] = slot_of[row[e_order]].astype(BF16)

    disout = np.zeros((TT, P), dtype=np.float32)
    node_of = np.full((TT, P), -1, dtype=np.int64)
    node_of[tile_of, slot_of] = np.arange(N)
    valid = node_of >= 0
    disout[valid] = dis[node_of[valid]]

    w16 = np.asarray(W, dtype=np.float32).astype(BF16)
    iota = np.tile(np.arange(P, dtype=np.float32).astype(BF16), (P, 1))

    in_maps = []
    for cidx in range(n_cores):
        sl = slice(cidx * n_tiles, (cidx + 1) * n_tiles)
        # device dlocal layout: [P(m), T*K] with column t*K+c
        dl = np.ascontiguousarray(
            dlocal[sl].reshape(n_tiles * K, P).T)
        do = np.ascontiguousarray(disout[sl].T)     # [P(slot), T]
        in_maps.append({
            "msgs": np.ascontiguousarray(
                msgs[sl].reshape(n_tiles, P, K * P)),
            "dlocal": dl,
            "disout": do,
            "w16": w16,
            "iota": iota,
        })
    return in_maps, node_of, n_tiles, K


def kernel(x, W, edge_index, num_nodes):
    N = int(num_nodes)
    in_maps, node_of, n_tiles, K = _route(x, W, edge_index, N)
    nc = build_nc(n_tiles, K)
    try:
        res = run_bass_kernel_spmd(nc, in_maps, core_ids=list(range(N_CORES)))
    except Exception:
        # a previous process can leave a core wedged (NRT_EXEC_UNIT_
        # UNRECOVERABLE); one retry after the runtime re-initializes
        # reliably clears it.
        import time as _time
        _time.sleep(5.0)
        res = run_bass_kernel_spmd(nc, in_maps, core_ids=list(range(N_CORES)))
    C = np.asarray(W).shape[1]
    out = np.zeros((N, C), dtype=np.float32)
    TT = node_of.shape[0]
    per_core = TT // N_CORES
    outs = np.concatenate(
        [res.results[c]["out"].reshape(per_core, P, C) for c in range(N_CORES)],
        axis=0)                                    # [TT, P, C]
    valid = node_of >= 0
    out[node_of[valid]] = outs[valid]
    return out



# revision 2
# speedup vs baseline: 5.8275x; 5.8275x over previous
"""GCNConv (N=100000, E=1600000, C=128) on 8 trn2 NeuronCores.

Node-parallel sharding (per the hint): destination nodes are partitioned
across the 8 cores; W is replicated; edges are routed by destination on
the host, which materializes each core's gathered source features as a
dest-major message stream (the hint's all-to-all happens in this host
routing step).

Layout: dests are sorted by message count (in-degree + self-loop) and
grouped into 128-dest tiles, so each tile has a uniform per-dest message
count J_t (even, zero-padded; ~4%% overhead). Messages are stored
TRANSPOSED per tile -- msgs[feat, j, dest] -- and quantized to int8 with
a global scale (clip 4.5 sigma) on the dis[col]-prescaled source rows;
the step is folded into W and dis[row] is applied by the host on the
returned output rows (it cancels in quantization SNR). rel err ~1.1e-2.

Device per tile (no per-message gather/selection at all):
  1. batched int8 DMA (sync HWDGE, ~1.5-2MB, tent-ordered tiles with
     ramped batch sizes for pipeline fill/drain),
  2. expansion int8->bf16 split across three engines by a min-makespan
     greedy: DVE / GpSimd do a halving-add (chunks j and j+J/2, exact in
     bf16), ACT does a whole-tile copy (PE then consumes full J),
  3. PE: psumT[fout, dest] += W.T-contraction(chunk) with W stationary
     (the segment-sum and the W transform fused into PSUM accumulation),
  4. one grouped ACT copy psum->SBUF bf16 per 8 tiles + out DMA on the
     ACT HWDGE ring.

Measured (neuron-profile): ~127 us per full pass on 8 cores; roofline is
the ~28MB/core int8 stream + ~3MB out at ~320GB/s with all three
expansion engines and DMA balanced at ~98us each.
"""
import math

import numpy as np
import ml_dtypes

import concourse.bacc as bacc
import concourse.tile as tile
from concourse import mybir
from concourse.bass_utils import run_bass_kernel_spmd

N_CORES = 8
P = 128

BF16 = ml_dtypes.bfloat16


def build_nc2(Js, batches, modes, deltas=None, out_group=8, repeat=1,
              msgs_cols=None, bufs=None, msg_mode="int8"):
    """Js: per-tile chunk counts (shared schedule, len T; even for int8).
    batches: list of (tile_start, tile_end) DMA batch ranges.
    modes[t]: expansion engine per tile — 'v' (DVE int8 TT-halving),
      'g' (GpSimd TT-halving), 'a' (ACT full-tile copy, PE does J matmuls).
    msg_mode: 'bf16' (no expansion, DVE bf16 halving) or 'int8'.
    msgs_cols (timing only): shrink msgs input, read modulo."""
    nc = bacc.Bacc("TRN2", target_bir_lowering=False, debug=False)
    f32 = mybir.dt.float32
    bf16 = mybir.dt.bfloat16
    T = len(Js)
    offs = np.zeros(T + 1, dtype=np.int64)
    np.cumsum(np.asarray(Js) * P, out=offs[1:])
    Wtot = int(offs[-1])

    b = {"msgp": 6, "sump": 6, "outp": 3, "ps": 4}
    if bufs:
        b.update(bufs)

    MC = msgs_cols if msgs_cols is not None else Wtot
    int8 = msg_mode == "int8"
    msg_dt = mybir.dt.int8 if int8 else bf16
    msgs = nc.dram_tensor("msgs", [P, MC], msg_dt, kind="ExternalInput")
    w16 = nc.dram_tensor("w16", [P, P], bf16, kind="ExternalInput")
    out = nc.dram_tensor("out", [P, T * P], bf16, kind="ExternalOutput")

    maxw = max(int(offs[e] - offs[s]) for s, e in batches)
    maxs = max(Js[t] if modes[t] == "a" else (Js[t] + 1) // 2
               for t in range(T))
    nog = out_group * P

    with tile.TileContext(nc) as tc:
        with tc.tile_pool(name="const", bufs=1) as constp, \
             tc.tile_pool(name="msgp", bufs=b["msgp"]) as msgp, \
             tc.tile_pool(name="sump", bufs=b["sump"]) as sump, \
             tc.tile_pool(name="outp", bufs=b["outp"]) as outp, \
             tc.tile_pool(name="ps", bufs=b["ps"], space="PSUM") as psp:
            w_t = constp.tile([P, P], bf16)
            nc.sync.dma_start(w_t[:], w16[:])

            def body():
                o_t = None
                o_base = 0
                for bs, be in batches:
                    w0 = int(offs[bs])
                    wid = int(offs[be] - offs[bs])
                    m_t = msgp.tile([P, maxw], msg_dt, tag="m")
                    src = msgs[:, w0:w0 + wid] if msgs_cols is None \
                        else msgs[:, 0:wid]
                    nc.sync.dma_start(m_t[:, :wid], src)
                    for t in range(bs, be):
                        J = int(Js[t])
                        base = int(offs[t]) - w0
                        gi = t % out_group
                        if gi == 0:
                            psb = psp.tile([P, nog], f32, tag="ps")
                            o_base = t
                        ps = psb[:, gi * P:(gi + 1) * P]
                        h = J // 2
                        mode = modes[t] if int8 else "v"
                        if not int8 and h == 0:
                            mode = "none"
                        if mode == "a":
                            s_t = sump.tile([P, maxs * P], bf16, tag="s")
                            nc.scalar.activation(
                                out=s_t[:, :J * P],
                                in_=m_t[:, base:base + J * P],
                                func=mybir.ActivationFunctionType.Copy)
                            nmm, s_src = J, s_t
                        elif mode != "none":
                            s_t = sump.tile([P, maxs * P], bf16, tag="s")
                            eng = nc.gpsimd if mode == "g" else nc.vector
                            eng.tensor_tensor(
                                out=s_t[:, :h * P],
                                in0=m_t[:, base:base + h * P],
                                in1=m_t[:, base + h * P:base + 2 * h * P],
                                op=mybir.AluOpType.add)
                            nmm, s_src = h + (J - 2 * h), s_t
                        if mode == "none":
                            for j in range(J):
                                nc.tensor.matmul(
                                    out=ps, lhsT=w_t[:],
                                    rhs=m_t[:, base + j * P:base + (j + 1) * P],
                                    start=(j == 0), stop=(j == J - 1))
                        else:
                            nj = nmm if mode == "a" else h
                            for j in range(nj):
                                nc.tensor.matmul(
                                    out=ps, lhsT=w_t[:],
                                    rhs=s_src[:, j * P:(j + 1) * P],
                                    start=(j == 0), stop=(j == nmm - 1))
                            if mode != "a" and J - 2 * h:
                                assert not int8
                                nc.tensor.matmul(
                                    out=ps, lhsT=w_t[:],
                                    rhs=m_t[:, base + (J - 1) * P:base + J * P],
                                    start=False, stop=True)
                        if gi == out_group - 1 or t == T - 1:
                            gw = (t + 1 - o_base) * P
                            o_t = outp.tile([P, nog], bf16, tag="o")
                            nc.scalar.copy(out=o_t[:, :gw], in_=psb[:, :gw])
                            nc.scalar.dma_start(
                                out[:, o_base * P:(t + 1) * P],
                                o_t[:, :gw])

            if repeat == 1:
                body()
            else:
                with tc.For_i(0, repeat, 1):
                    body()
    nc.compile()
    return nc


def _route2(x, W, edge_index, num_nodes, n_cores=N_CORES, maxw_cols=None,
            out_group=8, msg_mode="int8"):
    """Host-side routing. Returns (in_maps, node_of, Js, batches)."""
    int8 = msg_mode != "bf16"
    if maxw_cols is None:
        maxw_cols = 16384
    N = int(num_nodes)
    row = np.asarray(edge_index[0], dtype=np.int64)
    col = np.asarray(edge_index[1], dtype=np.int64)
    loops = np.arange(N, dtype=np.int64)
    row = np.concatenate([row, loops])
    col = np.concatenate([col, loops])
    E = row.shape[0]

    deg = np.bincount(col, minlength=N)
    dis = np.zeros(N, dtype=np.float32)
    nz = deg > 0
    dis[nz] = 1.0 / np.sqrt(deg[nz].astype(np.float64)).astype(np.float32)

    cnt = np.bincount(row, minlength=N)  # messages per dest (>=1)

    # pad dest set to full tiles: TT tiles of P dests, TT % n_cores == 0
    TT = math.ceil(N / P)
    TT = math.ceil(TT / n_cores) * n_cores
    T = TT // n_cores
    ND = TT * P
    cnt_p = np.zeros(ND, dtype=np.int64)
    cnt_p[:N] = cnt
    order = np.argsort(-cnt_p, kind="stable")  # dummies (cnt 0) sort last

    # global tile g holds order[g*P:(g+1)*P]; core g%n_cores, slot g//n_cores
    Jg = cnt_p[order].reshape(TT, P).max(axis=1)
    Js0 = np.maximum(Jg.reshape(T, n_cores).max(axis=1), 1)  # desc-J schedule
    if msg_mode == "int8":
        Js0 = (Js0 + 1) // 2 * 2           # even J: halving covers all chunks

    # tent schedule: small-J tiles at both ends (fast pipeline fill/drain),
    # big-J in the middle. perm[newpos] = old tile index (Js0 descending).
    asc = np.arange(T - 1, -1, -1)          # old idx, ascending J
    perm = np.concatenate([asc[0::2], asc[1::2][::-1]])  # up, peak, down
    pos_of = np.empty(T, dtype=np.int64)
    pos_of[perm] = np.arange(T)
    Js = Js0[perm]
    offs = np.zeros(T + 1, dtype=np.int64)
    np.cumsum(Js * P, out=offs[1:])
    Wtot = int(offs[-1])

    # DMA batches: consecutive tiles, ramped width caps at both ends
    start_caps = [1024, 2048, 4096]
    end_caps = [1024, 2048]            # final batches, smallest last
    tail = []
    e = T
    for cap in end_caps:
        s = e - 1
        while s > 0 and offs[e] - offs[s - 1] <= cap:
            s -= 1
        tail.append((int(s), int(e)))
        e = s
    tail.reverse()
    batches = []
    s = 0
    bi = 0
    while s < e:
        cap = start_caps[bi] if bi < len(start_caps) else maxw_cols
        t2 = s + 1
        while t2 < e and offs[t2 + 1] - offs[s] <= cap:
            t2 += 1
        batches.append((s, int(t2)))
        s = t2
        bi += 1
    batches += tail

    # per-dest placement
    gt_of = np.empty(ND, dtype=np.int64)   # global tile of dest
    sl_of = np.empty(ND, dtype=np.int64)   # slot within tile
    gt_of[order] = np.repeat(np.arange(TT), P)
    sl_of[order] = np.tile(np.arange(P), TT)
    core_of = gt_of % n_cores
    t_of = pos_of[gt_of // n_cores]        # tile slot in tent order

    # per-edge positions: j index within dest
    e_order = np.argsort(row, kind="stable")
    row_s = row[e_order]
    col_s = col[e_order]
    starts = np.searchsorted(row_s, np.arange(N))
    j_e = np.arange(E, dtype=np.int64) - starts[row_s]

    # column index within the core's msgs tensor
    colidx = offs[t_of[row_s]] + j_e * P + sl_of[row_s]
    core_e = core_of[row_s]

    x32 = np.asarray(x, dtype=np.float32) * dis[:, None]
    wf = np.asarray(W, dtype=np.float32)
    if int8:
        # global int8 scale on dis[col]-prescaled source rows; dis[row] is
        # applied by the HOST on the returned output rows (it cancels in
        # the quantization SNR), and the step is folded into W.
        xsq = (x32.astype(np.float64) ** 2).mean(axis=1)
        sig = float(np.sqrt(xsq[col_s].mean()))
        delta = 4.5 * sig / 127.0
        w16 = (wf * delta).astype(BF16)
    else:
        w16 = wf.astype(BF16)

    # per-tile expansion engine assignment (int8): greedy min-makespan over
    # DVE / GpSimd TT-halving, ACT whole-tile copy, and PE (shadow; 'a'
    # tiles cost full J matmuls). Rates measured from HW traces.
    modes = ["v"] * T
    if int8:
        EL = P * P / 1e3            # elems per chunk, in K
        load = {"v": 0.0, "g": 0.0, "a": 0.0, "pe": 0.0}
        for t in range(T):
            J = int(Js[t])
            h = J // 2
            cost = {
                "v": 0.20 + h * 0.315,
                "g": 0.25 + h * 0.335,
                "a": 0.25 + J * 0.1216,
            }
            pe_cost = {"v": h * 0.0764, "g": h * 0.0764, "a": J * 0.0764}
            load["a"] += 0.17       # grouped psum->out copy + dma on ACT
            load["pe"] += 0.0
            best, bestmax = None, None
            for m in ("v", "g", "a"):
                mx = max(load["v"] + (cost["v"] if m == "v" else 0),
                         load["g"] + (cost["g"] if m == "g" else 0),
                         load["a"] + (cost["a"] if m == "a" else 0),
                         load["pe"] + pe_cost[m])
                if bestmax is None or mx < bestmax:
                    best, bestmax = m, mx
            modes[t] = best
            load[best] += cost[best]
            load["pe"] += pe_cost[best]

    in_maps = []
    for c in range(n_cores):
        m = core_e == c
        vals32 = x32[col_s[m]]
        if int8:
            q = np.clip(np.rint(vals32 / delta), -127, 127).astype(np.int8)
            arr = np.zeros((Wtot, P), dtype=np.int8)
            arr[colidx[m]] = q
        else:
            arr = np.zeros((Wtot, P), dtype=BF16)
            arr[colidx[m]] = vals32.astype(BF16)
        in_maps.append({
            "msgs": np.ascontiguousarray(arr.T),
            "w16": w16,
        })

    node_of = np.full((n_cores, T, P), -1, dtype=np.int64)
    src = order.reshape(TT, P)
    for c in range(n_cores):
        node_of[c] = src[c::n_cores][perm]
    node_of[node_of >= N] = -1

    return in_maps, node_of, [int(j) for j in Js], batches, modes, dis


def kernel(x, W, edge_index, num_nodes):
    N = int(num_nodes)
    in_maps, node_of, Js, batches, modes, dis = _route2(x, W, edge_index, N)
    nc = build_nc2(Js, batches, modes)
    try:
        res = run_bass_kernel_spmd(nc, in_maps, core_ids=list(range(N_CORES)))
    except Exception:
        import time as _time
        _time.sleep(5.0)
        res = run_bass_kernel_spmd(nc, in_maps, core_ids=list(range(N_CORES)))
    C = np.asarray(W).shape[1]
    T = len(Js)
    out = np.zeros((N, C), dtype=np.float32)
    for c in range(N_CORES):
        o = res.results[c]["out"].astype(np.float32)      # [fout, T*P]
        o = o.reshape(C, T, P).transpose(1, 2, 0)          # [T, dest, fout]
        nof = node_of[c]
        valid = nof >= 0
        nodes = nof[valid]
        out[nodes] = o[valid] * dis[nodes][:, None]
    return out
